# revision 3
# baseline (speedup 1.0000x reference)
"""Trainium2 Bass kernel for a single decoder block (B=2, T=2048, C=1024,
NH=16, DFF=4096), distributed over 8 NeuronCores.

Head-parallel attention + token-parallel FFN (see kernel_v2 docstring).
v5: denominator reciprocals on partition-major [8,512] tiles (2 DVE
calls instead of 16 broadcast-sized ones), PE-broadcast of 1/den,
half-a normalization overlapped under the second AllToAll, LN chains
split across DVE/Pool by feature-tile parity, deeper score psum ring,
8-way x load. Also v4: per-head zero-padded q tiles so every attention matmul contracts
over the full 128 partitions (uniform PE tile config -> pipelined
matmuls), AllToAll split per head half with the first overlapped under
the second half's attention compute. Also v3: quad-query attention tiles (512-wide matmuls with exact causal
trim), 2-key-tile batched exp, diagonal masks on the Pool engine,
v-projection feature-major + PE transpose, feature-major final LN with
stats accumulated inside the FFN2 loop, elementwise LN chains split
across DVE and Pool, token-blocked x loads. Output is feature-major
[C, 512] per core; the host transposes.
"""

import sys

if "/opt/trn_rl_repo" not in sys.path:
    sys.path.insert(0, "/opt/trn_rl_repo")

from contextlib import ExitStack

import numpy as np
import ml_dtypes

B, T, C = 2, 2048, 1024
NH, HD, DFF = 16, 64, 4096
N_CORES = 8
NT = B * T            # 4096 flat tokens
TCH = 512             # tokens per core (FFN phase)
NCT = C // 128        # 8 feature tiles
NQT = T // 128        # 16 query/key tiles per batch
SCALE = 1.0 / 32.0    # 1/sqrt(C)
EPS = 1e-5

_CACHE = {}


def _build(collective=True):
    import concourse.mybir as mybir
    import concourse.tile as tile
    from concourse import bacc

    F32 = mybir.dt.float32
    F32R = mybir.dt.float32r
    F16 = mybir.dt.float16
    BF16 = mybir.dt.bfloat16
    AF = mybir.ActivationFunctionType
    ALU = mybir.AluOpType

    nc = bacc.Bacc("TRN2", target_bir_lowering=False, debug=False,
                   num_devices=N_CORES)

    def din(name, shape):
        return nc.dram_tensor(name, shape, F32, kind="ExternalInput").ap()

    def dinb(name, shape):
        return nc.dram_tensor(name, shape, BF16, kind="ExternalInput").ap()

    xt = dinb("xt", [C, NT])
    wq = dinb("wq", [C, 128])
    wk = dinb("wk", [C, 128])
    wv = dinb("wv", [C, 128])
    w1 = dinb("w1", [C, DFF])
    w2 = dinb("w2", [DFF, C])
    bqkv = din("bqkv", [128, 3])
    b1_r = din("b1_r", [128, DFF // 128])
    b2_r = din("b2_r", [128, NCT])
    lnw_r = din("lnw_r", [128, NCT])
    ident_in = din("ident_in", [128, 128])
    ones_in = din("ones_in", [128, 128])
    diag_in = nc.dram_tensor("diag_in", [128, 128], BF16,
                             kind="ExternalInput").ap()
    pick8a_in = din("pick8a_in", [8, NCT, 128])
    pick8b_in = din("pick8b_in", [8, NCT, 128])
    outc = nc.dram_tensor("outc", [C, TCH], F32, kind="ExternalOutput").ap()

    with tile.TileContext(nc) as tc, ExitStack() as S0, \
            nc.allow_low_precision(reason="bf16/fp16 matmul operand rounding"):
      persist = S0.enter_context(tc.tile_pool(name="persist", bufs=1))
      dram = S0.enter_context(tc.tile_pool(name="dram", bufs=1, space="DRAM"))

      # ---- constants
      ones_all = persist.tile([128, 128], F32R)
      nc.sync.dma_start(ones_all, ones_in.bitcast(F32R))
      eps_sb = persist.tile([128, 1], F32)
      nc.vector.memset(eps_sb, EPS)
      ones16 = persist.tile([128, 1], F16)
      nc.vector.memset(ones16, 1.0)
      diag_sb = persist.tile([128, 128], BF16)
      nc.sync.dma_start(diag_sb, diag_in)
      pick8a = persist.tile([8, NCT, 128], F32R)
      nc.sync.dma_start(pick8a, pick8a_in.bitcast(F32R))
      pick8b = persist.tile([8, NCT, 128], F32R)
      nc.sync.dma_start(pick8b, pick8b_in.bitcast(F32R))
      bqkv_sb = persist.tile([128, 3], F32)
      nc.sync.dma_start(bqkv_sb, bqkv)
      ident_f = persist.tile([128, 128], F32)
      nc.sync.dma_start(ident_f, ident_in)
      ident_bf = persist.tile([128, 128], BF16)
      nc.vector.tensor_copy(ident_bf, ident_f)
      b1_sb = persist.tile([128, DFF // 128], F32)
      nc.sync.dma_start(b1_sb, b1_r)
      b2_sb = persist.tile([128, NCT], F32)
      nc.sync.dma_start(b2_sb, b2_r)
      lnw_sb = persist.tile([128, NCT], F32)
      nc.sync.dma_start(lnw_sb, lnw_r)

      # ---- AllToAll buffers, one per head half: [65, 512] fp16 per dest
      # (rows 0:64 = unnormalized head_out features, row 64 = denominator)
      a2a_in1 = dram.tile([N_CORES, 65, TCH], F16)
      a2a_out1 = dram.tile([N_CORES, 65, TCH], F16)
      a2a_in2 = dram.tile([N_CORES, 65, TCH], F16)
      a2a_out2 = dram.tile([N_CORES, 65, TCH], F16)

      with ExitStack() as SATT:
          qkp = SATT.enter_context(tc.tile_pool(name="qkp", bufs=1))
          q2a = qkp.tile([128, NT], BF16)   # head 0 feats in rows 0:64, rows 64:128 zero
          q2b = qkp.tile([128, NT], BF16)   # head 1 feats in rows 64:128, rows 0:64 zero
          k2 = qkp.tile([128, NT], BF16)
          vaug = qkp.tile([128, NT // 128, 2, HD + 1], BF16)

          # ================= Phase 1: q/k/v projections ==================
          with ExitStack() as S1:
              xtp = S1.enter_context(tc.tile_pool(name="xtp", bufs=1))
              wpp = S1.enter_context(tc.tile_pool(name="wpp", bufs=1))
              vtp = S1.enter_context(tc.tile_pool(name="vtp", bufs=2))
              qkps = S1.enter_context(tc.tile_pool(name="qkps", bufs=2,
                                                   space="PSUM"))
              vtps = S1.enter_context(tc.tile_pool(name="vtps", bufs=2,
                                                   space="PSUM"))

              wk_sb = wpp.tile([128, NCT, 128], BF16)
              nc.sync.dma_start(wk_sb,
                                wk.rearrange("(ci r) o -> r ci o", r=128))
              wq_sb = wpp.tile([128, NCT, 128], BF16)
              nc.sync.dma_start(wq_sb,
                                wq.rearrange("(ci r) o -> r ci o", r=128))
              wv_sb = wpp.tile([128, NCT, 128], BF16)
              nc.sync.dma_start(wv_sb,
                                wv.rearrange("(ci r) o -> r ci o", r=128))

              xT = xtp.tile([128, NCT, NT], BF16)
              xr = xt.rearrange("(ci p) t -> p ci t", p=128)
              for g in range(8):
                  gs = slice(g * 512, (g + 1) * 512)
                  nc.sync.dma_start(xT[:, :, gs], xr[:, :, gs])

              # ones column of v_aug (softmax denominator accumulator)
              nc.vector.memset(vaug[:, :, :, HD:HD + 1], 1.0)
              nc.vector.memset(q2a[HD:128, :], 0.0)
              nc.vector.memset(q2b[0:HD, :], 0.0)

              for tt in range(NT // TCH):
                  ts = slice(tt * TCH, (tt + 1) * TCH)
                  ps = qkps.tile([128, TCH], F32, tag="qkp")
                  for ci in range(NCT):
                      nc.tensor.matmul(ps, wk_sb[:, ci, :], xT[:, ci, ts],
                                       start=(ci == 0), stop=(ci == NCT - 1))
                  nc.scalar.activation(k2[:, ts], ps, AF.Identity,
                                       bias=bqkv_sb[:, 1:2])
                  ps = qkps.tile([128, TCH], F32, tag="qkp")
                  for ci in range(NCT):
                      nc.tensor.matmul(ps, wq_sb[:, ci, :], xT[:, ci, ts],
                                       start=(ci == 0), stop=(ci == NCT - 1))
                  nc.scalar.activation(q2a[0:HD, ts], ps[0:HD, :], AF.Identity,
                                       bias=bqkv_sb[0:HD, 0:1])
                  nc.scalar.activation(q2b[HD:128, ts], ps[HD:128, :],
                                       AF.Identity, bias=bqkv_sb[HD:128, 0:1])
                  # v: feature-major matmul, bias, then PE transpose into
                  # the token-major v_aug layout
                  ps = qkps.tile([128, TCH], F32, tag="qkp")
                  for ci in range(NCT):
                      nc.tensor.matmul(ps, wv_sb[:, ci, :], xT[:, ci, ts],
                                       start=(ci == 0), stop=(ci == NCT - 1))
                  vt = vtp.tile([128, TCH], BF16, tag="vt")
                  nc.scalar.activation(vt, ps, AF.Identity,
                                       bias=bqkv_sb[:, 2:3])
                  for u in range(4):
                      kt = 4 * tt + u
                      tp = vtps.tile([128, 128], BF16, tag="tp")
                      nc.tensor.transpose(
                          tp, vt[:, u * 128:(u + 1) * 128], ident_bf)
                      nc.vector.tensor_copy(
                          vaug[:, kt, :, 0:HD],
                          tp[:].rearrange("p (h f) -> p h f", f=HD))

          # ================= Phase 2: causal attention ===================
          with ExitStack() as S3:
              pup = S3.enter_context(tc.tile_pool(name="pup", bufs=3))
              avsp = S3.enter_context(tc.tile_pool(name="avsp", bufs=3))
              stps = S3.enter_context(tc.tile_pool(name="stps", bufs=3,
                                                   space="PSUM"))
              avps = S3.enter_context(tc.tile_pool(name="avps", bufs=2,
                                                   space="PSUM"))

              for hl in range(2):
                  qh = q2a if hl == 0 else q2b
                  a2a_i = a2a_in1 if hl == 0 else a2a_in2
                  for b in range(B):
                      for j in range(4):  # quads of 4 query tiles
                          qb = b * T + j * 512
                          nki = 4 * j + 4
                          av = avps.tile([HD + 1, TCH], F32, tag="av")
                          for p in range(nki // 2):
                              ki0, ki1 = 2 * p, 2 * p + 1
                              su = max(0, ki0 - 4 * j) * 128
                              st = stps.tile([128, 2, TCH], F32, tag="st")
                              for sl, ki in ((0, ki0), (1, ki1)):
                                  # write from the pair's union start so the
                                  # batched exp below never reads stale PSUM
                                  # (exp(garbage) can be inf/NaN)
                                  ks = slice(b * T + ki * 128,
                                             b * T + ki * 128 + 128)
                                  nc.tensor.matmul(
                                      st[:, sl, su:TCH],
                                      k2[:, ks],
                                      qh[:, qb + su:qb + TCH],
                                      start=True, stop=True)
                              pu = pup.tile([128, 2, TCH], BF16, tag="pu")
                              nc.scalar.activation(pu[:, :, su:TCH],
                                                   st[:, :, su:TCH],
                                                   AF.Exp, scale=SCALE)
                              for sl, ki in ((0, ki0), (1, ki1)):
                                  if ki >= 4 * j:  # diagonal tile
                                      s = (ki - 4 * j) * 128
                                      nc.gpsimd.tensor_mul(
                                          pu[:, sl, s:s + 128],
                                          pu[:, sl, s:s + 128], diag_sb)
                              for sl, ki in ((0, ki0), (1, ki1)):
                                  s = max(0, ki - 4 * j) * 128
                                  nc.tensor.matmul(
                                      av[:, s:TCH],
                                      vaug[:, b * NQT + ki, hl, :],
                                      pu[:, sl, s:TCH],
                                      start=(ki == 0), stop=(ki == nki - 1),
                                      skip_group_check=True)
                          avs = avsp.tile([HD + 1, TCH], F16, tag="avs")
                          nc.vector.tensor_copy(avs, av)
                          d = 4 * b + j
                          nc.sync.dma_start(a2a_i[d, :, :], avs)
                  if collective:
                      nc.gpsimd.collective_compute(
                          "AllToAll", mybir.AluOpType.bypass,
                          replica_groups=[list(range(N_CORES))],
                          ins=[(a2a_in1 if hl == 0 else a2a_in2)[:].opt()],
                          outs=[(a2a_out1 if hl == 0 else a2a_out2)[:].opt()])
                  else:
                      nc.sync.dma_start(
                          (a2a_out1 if hl == 0 else a2a_out2)[:],
                          (a2a_in1 if hl == 0 else a2a_in2)[:])

      # ================= Phase 3: normalize + h = a + LN(a) =============
      with ExitStack() as SH:
          hp = SH.enter_context(tc.tile_pool(name="hp", bufs=1))
          aT = hp.tile([128, NCT, TCH], F16)
          hT = hp.tile([128, NCT, TCH], BF16)
          fT = hp.tile([128, NCT, TCH], F32R)

          with ExitStack() as S4:
              arp = S4.enter_context(tc.tile_pool(name="arp", bufs=1))
              sqp = S4.enter_context(tc.tile_pool(name="sqp", bufs=3))
              stsb = S4.enter_context(tc.tile_pool(name="stsb", bufs=1))
              dnps = S4.enter_context(tc.tile_pool(name="dnps", bufs=2,
                                                   space="PSUM"))
              smps = S4.enter_context(tc.tile_pool(name="smps", bufs=1,
                                                   space="PSUM"))

              araw = arp.tile([128, NCT, TCH], F16)
              sum_ps = smps.tile([1, TCH], F32, tag="sma")
              sq_ps = smps.tile([1, TCH], F32, tag="smb")
              rcp8 = [None, None]
              # half 0 depends only on the first AllToAll and overlaps the
              # second; half 1 follows once a2a_out2 lands
              for h, a2o in ((0, a2a_out1), (1, a2a_out2)):
                  hs = slice(64 * h, 64 * h + 64)
                  nc.sync.dma_start(
                      araw[hs, :, :],
                      a2o[:, 0:HD, :].rearrange("c p t -> p c t"))
                  den8 = arp.tile([NCT, TCH], F16, tag=f"d8{h}")
                  nc.sync.dma_start(
                      den8, a2o[:, HD:HD + 1, :].rearrange("c p t -> (c p) t"))
                  r8 = arp.tile([NCT, TCH], F32R, tag=f"r8{h}")
                  nc.vector.reciprocal(r8, den8)
                  rcp8[h] = r8
              for h in range(2):
                  hs = slice(64 * h, 64 * h + 64)
                  pick = pick8a if h == 0 else pick8b
                  for ci in range(NCT):
                      dbc = dnps.tile([128, TCH], F32, tag="dbc")
                      nc.tensor.matmul(dbc, pick[:, ci, :], rcp8[h],
                                       start=True, stop=True)
                      nc.vector.tensor_mul(aT[hs, ci, :], araw[hs, ci, :],
                                           dbc[hs, :])
                      nc.tensor.matmul(sum_ps, ones16[hs, :], aT[hs, ci, :],
                                       start=(h == 0 and ci == 0),
                                       stop=(h == 1 and ci == NCT - 1),
                                       skip_group_check=True)
                      asq = sqp.tile([128, TCH], F32R, tag="asq")
                      nc.scalar.activation(asq[hs, :], aT[hs, ci, :],
                                           AF.Square)
                      nc.tensor.matmul(sq_ps, ones_all[hs, 0:1], asq[hs, :],
                                       start=(h == 0 and ci == 0),
                                       stop=(h == 1 and ci == NCT - 1),
                                       skip_group_check=True)
              mu_sb = stsb.tile([1, TCH], F32R, tag="s1")
              nc.vector.tensor_scalar_mul(mu_sb, sum_ps, 1.0 / C)
              ex2 = stsb.tile([1, TCH], F32, tag="s2")
              nc.vector.tensor_scalar_mul(ex2, sq_ps, 1.0 / C)
              musq = stsb.tile([1, TCH], F32, tag="s3")
              nc.vector.tensor_mul(musq, mu_sb, mu_sb)
              var = stsb.tile([1, TCH], F32, tag="s4")
              nc.vector.tensor_sub(var, ex2, musq)
              sd = stsb.tile([1, TCH], F32, tag="s5")
              nc.scalar.activation(sd, var, AF.Sqrt, bias=eps_sb[0:1, :])
              rs_sb = stsb.tile([1, TCH], F32R, tag="s6")
              nc.vector.reciprocal(rs_sb, sd)
              mu_bc = dnps.tile([128, TCH], F32, tag="dbc")
              nc.tensor.matmul(mu_bc, ones_all[0:1, :], mu_sb[:],
                               start=True, stop=True)
              rs_bc = dnps.tile([128, TCH], F32, tag="dbc")
              nc.tensor.matmul(rs_bc, ones_all[0:1, :], rs_sb[:],
                               start=True, stop=True)
              mu_bs = sqp.tile([128, TCH], F32, tag="mbs")
              nc.vector.tensor_copy(mu_bs, mu_bc)
              rs_bs = sqp.tile([128, TCH], F32, tag="rbs")
              nc.vector.tensor_copy(rs_bs, rs_bc)
              for ci in range(NCT):
                  t1 = sqp.tile([128, TCH], F32, tag="t1")
                  nc.gpsimd.tensor_sub(t1, aT[:, ci, :], mu_bs)
                  t2 = sqp.tile([128, TCH], F32, tag="t2")
                  nc.gpsimd.tensor_mul(t2, t1, rs_bs)
                  nc.vector.scalar_tensor_tensor(
                      out=hT[:, ci, :], in0=t2,
                      scalar=lnw_sb[:, ci:ci + 1], in1=aT[:, ci, :],
                      op0=ALU.mult, op1=ALU.add)

          # ================= Phase 4/5: FFN (+ final LN stats) ==========
          with ExitStack() as S5:
              gp = S5.enter_context(tc.tile_pool(name="gp", bufs=1))
              w1p = S5.enter_context(tc.tile_pool(name="w1p", bufs=3))
              w2p = S5.enter_context(tc.tile_pool(name="w2p", bufs=2))
              sq7p = S5.enter_context(tc.tile_pool(name="sq7p", bufs=2))
              st7 = S5.enter_context(tc.tile_pool(name="st7", bufs=1))
              ffps = S5.enter_context(tc.tile_pool(name="ffps", bufs=3,
                                                   space="PSUM"))
              sm7ps = S5.enter_context(tc.tile_pool(name="sm7ps", bufs=1,
                                                    space="PSUM"))

              gT = gp.tile([128, DFF // 128, TCH], BF16)
              for mt in range(DFF // 128):
                  wt = w1p.tile([128, NCT, 128], BF16, tag="w1t")
                  nc.sync.dma_start(
                      wt, w1.rearrange("(ci r) f -> r ci f", r=128)
                      [:, :, mt * 128:(mt + 1) * 128])
                  ps = ffps.tile([128, TCH], F32, tag="f1")
                  for ci in range(NCT):
                      nc.tensor.matmul(ps, wt[:, ci, :], hT[:, ci, :],
                                       start=(ci == 0), stop=(ci == NCT - 1))
                  nc.scalar.activation(gT[:, mt, :], ps, AF.Relu,
                                       bias=b1_sb[:, mt:mt + 1])

              sum7 = sm7ps.tile([1, TCH], F32, tag="sm7a")
              sq7 = sm7ps.tile([1, TCH], F32, tag="sm7b")
              for ci in range(NCT):
                  wt = w2p.tile([128, DFF // 128, 128], BF16, tag="w2t")
                  nc.sync.dma_start(
                      wt, w2.rearrange("(gk r) f -> r gk f", r=128)
                      [:, :, ci * 128:(ci + 1) * 128])
                  ps = ffps.tile([128, TCH], F32, tag="f2")
                  for gk in range(DFF // 128):
                      nc.tensor.matmul(ps, wt[:, gk, :], gT[:, gk, :],
                                       start=(gk == 0),
                                       stop=(gk == DFF // 128 - 1))
                  nc.scalar.activation(fT[:, ci, :], ps, AF.Identity,
                                       bias=b2_sb[:, ci:ci + 1])
                  nc.tensor.matmul(sum7, ones_all[:, 0:1], fT[:, ci, :],
                                   start=(ci == 0), stop=(ci == NCT - 1))
                  asq7 = sq7p.tile([128, TCH], F32R, tag="asq7")
                  nc.scalar.activation(asq7, fT[:, ci, :], AF.Square)
                  nc.tensor.matmul(sq7, ones_all[:, 0:1], asq7[:],
                                   start=(ci == 0), stop=(ci == NCT - 1))

              # ---- final LN stats + out = f + LN(f), feature-major
              with ExitStack() as S7:
                  op7 = S7.enter_context(tc.tile_pool(name="op7", bufs=2))

                  mu7 = st7.tile([1, TCH], F32R, tag="m7")
                  nc.vector.tensor_scalar_mul(mu7, sum7, 1.0 / C)
                  ex27 = st7.tile([1, TCH], F32, tag="e7")
                  nc.vector.tensor_scalar_mul(ex27, sq7, 1.0 / C)
                  musq7 = st7.tile([1, TCH], F32, tag="mq7")
                  nc.vector.tensor_mul(musq7, mu7, mu7)
                  var7 = st7.tile([1, TCH], F32, tag="v7")
                  nc.vector.tensor_sub(var7, ex27, musq7)
                  sd7 = st7.tile([1, TCH], F32, tag="sd7")
                  nc.scalar.activation(sd7, var7, AF.Sqrt,
                                       bias=eps_sb[0:1, :])
                  rs7 = st7.tile([1, TCH], F32R, tag="rs7")
                  nc.vector.reciprocal(rs7, sd7)
                  mu7_bc = ffps.tile([128, TCH], F32, tag="f2")
                  nc.tensor.matmul(mu7_bc, ones_all[0:1, :], mu7[:],
                                   start=True, stop=True)
                  rs7_bc = ffps.tile([128, TCH], F32, tag="f2")
                  nc.tensor.matmul(rs7_bc, ones_all[0:1, :], rs7[:],
                                   start=True, stop=True)
                  mu7_bs = op7.tile([128, TCH], F32, tag="m7s")
                  nc.vector.tensor_copy(mu7_bs, mu7_bc)
                  rs7_bs = op7.tile([128, TCH], F32, tag="r7s")
                  nc.vector.tensor_copy(rs7_bs, rs7_bc)
                  ocr = outc.rearrange("(ci p) t -> p ci t", p=128)
                  for ci in range(NCT):
                      t1 = op7.tile([128, TCH], F32, tag="t17")
                      nc.gpsimd.tensor_sub(t1, fT[:, ci, :], mu7_bs)
                      t2 = op7.tile([128, TCH], F32, tag="t27")
                      nc.gpsimd.tensor_mul(t2, t1, rs7_bs)
                      ot = op7.tile([128, TCH], F32, tag="ot7")
                      nc.vector.scalar_tensor_tensor(
                          out=ot, in0=t2,
                          scalar=lnw_sb[:, ci:ci + 1], in1=fT[:, ci, :],
                          op0=ALU.mult, op1=ALU.add)
                      nc.sync.dma_start(ocr[:, ci, :], ot)

    nc.compile()
    return nc


def _stage(inputs):
    bf = ml_dtypes.bfloat16
    f16 = np.float16
    x = np.asarray(inputs["x"], dtype=np.float32)
    xt = np.ascontiguousarray(x.reshape(NT, C).T.astype(bf))
    Wq = np.asarray(inputs["Wq"], np.float32)
    Wk = np.asarray(inputs["Wk"], np.float32)
    Wv = np.asarray(inputs["Wv"], np.float32)
    bq = np.asarray(inputs["bq"], np.float32)
    bk = np.asarray(inputs["bk"], np.float32)
    bv = np.asarray(inputs["bv"], np.float32)

    diag = np.triu(np.ones((128, 128), np.float32)).astype(bf)
    pick8a = np.zeros((8, NCT, 128), np.float32)
    pick8b = np.zeros((8, NCT, 128), np.float32)
    for ci in range(NCT):
        pick8a[ci, ci, 0:64] = 1.0
        pick8b[ci, ci, 64:128] = 1.0

    shared = {
        "xt": xt,
        "w1": np.ascontiguousarray(np.asarray(inputs["W1"], np.float32).astype(bf)),
        "w2": np.ascontiguousarray(np.asarray(inputs["W2"], np.float32).astype(bf)),
        "b1_r": np.ascontiguousarray(
            np.asarray(inputs["b1"], np.float32).reshape(DFF // 128, 128).T),
        "b2_r": np.ascontiguousarray(
            np.asarray(inputs["b2"], np.float32).reshape(NCT, 128).T),
        "lnw_r": np.ascontiguousarray(
            np.asarray(inputs["ln_w"], np.float32).reshape(NCT, 128).T),
        "ident_in": np.eye(128, dtype=np.float32),
        "ones_in": np.ones((128, 128), dtype=np.float32),
        "diag_in": np.ascontiguousarray(diag),
        "pick8a_in": pick8a,
        "pick8b_in": pick8b,
    }
    in_maps = []
    for c in range(N_CORES):
        sl = slice(128 * c, 128 * c + 128)
        per = dict(shared)
        per["wq"] = np.ascontiguousarray(Wq[:, sl].astype(bf))
        per["wk"] = np.ascontiguousarray(Wk[:, sl].astype(bf))
        per["wv"] = np.ascontiguousarray(Wv[:, sl].astype(bf))
        per["bqkv"] = np.ascontiguousarray(
            np.stack([bq[sl], bk[sl], bv[sl]], axis=1).astype(np.float32))
        in_maps.append(per)
    return in_maps


def kernel(**inputs):
    from concourse.bass_utils import run_bass_kernel_spmd

    nc = _CACHE.get("nc")
    if nc is None:
        nc = _CACHE["nc"] = _build()
    in_maps = _stage(inputs)
    res = run_bass_kernel_spmd(nc, in_maps, core_ids=list(range(N_CORES)))
    out = np.empty((B, T, C), dtype=np.float32)
    for c in range(N_CORES):
        b, m = divmod(c, 4)
        out[b, m * TCH:(m + 1) * TCH, :] = res.results[c]["outc"].T
    return out


# revision 4
# speedup vs baseline: 1.0197x; 1.0197x over previous
"""Trainium2 Bass kernel for a single decoder block (B=2, T=2048, C=1024,
NH=16, DFF=4096), distributed over 8 NeuronCores.

Head-parallel attention + token-parallel FFN (see kernel_v2 docstring).
v5: denominator reciprocals on partition-major [8,512] tiles (2 DVE
calls instead of 16 broadcast-sized ones), PE-broadcast of 1/den,
half-a normalization overlapped under the second AllToAll, LN chains
split across DVE/Pool by feature-tile parity, deeper score psum ring,
8-way x load. Also v4: per-head zero-padded q tiles so every attention matmul contracts
over the full 128 partitions (uniform PE tile config -> pipelined
matmuls), AllToAll split per head half with the first overlapped under
the second half's attention compute. Also v3: quad-query attention tiles (512-wide matmuls with exact causal
trim), 2-key-tile batched exp, diagonal masks on the Pool engine,
v-projection feature-major + PE transpose, feature-major final LN with
stats accumulated inside the FFN2 loop, elementwise LN chains split
across DVE and Pool, token-blocked x loads. Output is feature-major
[C, 512] per core; the host transposes.
"""

import sys

if "/opt/trn_rl_repo" not in sys.path:
    sys.path.insert(0, "/opt/trn_rl_repo")

from contextlib import ExitStack

import numpy as np
import ml_dtypes

B, T, C = 2, 2048, 1024
NH, HD, DFF = 16, 64, 4096
N_CORES = 8
NT = B * T            # 4096 flat tokens
TCH = 512             # tokens per core (FFN phase)
NCT = C // 128        # 8 feature tiles
NQT = T // 128        # 16 query/key tiles per batch
SCALE = 1.0 / 32.0    # 1/sqrt(C)
EPS = 1e-5

_CACHE = {}


def _build(collective=True):
    import concourse.mybir as mybir
    import concourse.tile as tile
    from concourse import bacc

    F32 = mybir.dt.float32
    F32R = mybir.dt.float32r
    F16 = mybir.dt.float16
    BF16 = mybir.dt.bfloat16
    AF = mybir.ActivationFunctionType
    ALU = mybir.AluOpType

    nc = bacc.Bacc("TRN2", target_bir_lowering=False, debug=False,
                   num_devices=N_CORES)

    def din(name, shape):
        return nc.dram_tensor(name, shape, F32, kind="ExternalInput").ap()

    def dinb(name, shape):
        return nc.dram_tensor(name, shape, BF16, kind="ExternalInput").ap()

    xt = dinb("xt", [C, NT])
    wq = dinb("wq", [C, 128])
    wk = dinb("wk", [C, 128])
    wv = dinb("wv", [C, 128])
    w1 = dinb("w1", [C, DFF])
    w2 = dinb("w2", [DFF, C])
    bqkv = din("bqkv", [128, 3])
    b1_r = din("b1_r", [128, DFF // 128])
    b2_r = din("b2_r", [128, NCT])
    lnw_r = din("lnw_r", [128, NCT])
    ident_in = din("ident_in", [128, 128])
    ones_in = din("ones_in", [128, 128])
    diag_in = nc.dram_tensor("diag_in", [128, 128], BF16,
                             kind="ExternalInput").ap()
    pick8a_in = din("pick8a_in", [8, NCT, 128])
    pick8b_in = din("pick8b_in", [8, NCT, 128])
    outc = nc.dram_tensor("outc", [C, TCH], F32, kind="ExternalOutput").ap()

    with tile.TileContext(nc) as tc, ExitStack() as S0, \
            nc.allow_low_precision(reason="bf16/fp16 matmul operand rounding"):
      persist = S0.enter_context(tc.tile_pool(name="persist", bufs=1))
      dram = S0.enter_context(tc.tile_pool(name="dram", bufs=1, space="DRAM"))

      # ---- constants
      ones_all = persist.tile([128, 128], F32R)
      nc.sync.dma_start(ones_all, ones_in.bitcast(F32R))
      eps_sb = persist.tile([128, 1], F32)
      nc.vector.memset(eps_sb, EPS)
      ones16 = persist.tile([128, 1], F16)
      nc.vector.memset(ones16, 1.0)
      diag_sb = persist.tile([128, 128], BF16)
      nc.sync.dma_start(diag_sb, diag_in)
      pick8a = persist.tile([8, NCT, 128], F32R)
      nc.sync.dma_start(pick8a, pick8a_in.bitcast(F32R))
      pick8b = persist.tile([8, NCT, 128], F32R)
      nc.sync.dma_start(pick8b, pick8b_in.bitcast(F32R))
      bqkv_sb = persist.tile([128, 3], F32)
      nc.sync.dma_start(bqkv_sb, bqkv)
      ident_f = persist.tile([128, 128], F32)
      nc.sync.dma_start(ident_f, ident_in)
      ident_bf = persist.tile([128, 128], BF16)
      nc.vector.tensor_copy(ident_bf, ident_f)
      b1_sb = persist.tile([128, DFF // 128], F32)
      nc.sync.dma_start(b1_sb, b1_r)
      b2_sb = persist.tile([128, NCT], F32)
      nc.sync.dma_start(b2_sb, b2_r)
      lnw_sb = persist.tile([128, NCT], F32)
      nc.sync.dma_start(lnw_sb, lnw_r)

      # ---- AllToAll buffers, one per head half: [65, 512] fp16 per dest
      # (rows 0:64 = unnormalized head_out features, row 64 = denominator)
      a2a_in1 = dram.tile([N_CORES, 65, TCH], F16)
      a2a_out1 = dram.tile([N_CORES, 65, TCH], F16)
      a2a_in2 = dram.tile([N_CORES, 65, TCH], F16)
      a2a_out2 = dram.tile([N_CORES, 65, TCH], F16)

      with ExitStack() as SATT:
          qkp = SATT.enter_context(tc.tile_pool(name="qkp", bufs=1))
          q2a = qkp.tile([128, NT], BF16)   # head 0 feats in rows 0:64, rows 64:128 zero
          q2b = qkp.tile([128, NT], BF16)   # head 1 feats in rows 64:128, rows 0:64 zero
          k2 = qkp.tile([128, NT], BF16)
          vaug = qkp.tile([128, NT // 128, 2, HD + 1], BF16)

          # ================= Phase 1: q/k/v projections ==================
          with ExitStack() as S1:
              xtp = S1.enter_context(tc.tile_pool(name="xtp", bufs=1))
              wpp = S1.enter_context(tc.tile_pool(name="wpp", bufs=1))
              vtp = S1.enter_context(tc.tile_pool(name="vtp", bufs=2))
              qkps = S1.enter_context(tc.tile_pool(name="qkps", bufs=2,
                                                   space="PSUM"))
              vtps = S1.enter_context(tc.tile_pool(name="vtps", bufs=2,
                                                   space="PSUM"))

              wmps = S1.enter_context(tc.tile_pool(name="wmps", bufs=1,
                                                   space="PSUM"))
              warm = wmps.tile([128, 128], F32, tag="wm")
              for _ in range(12):
                  nc.tensor.matmul(warm, ones_all, ones_all,
                                   start=True, stop=True)

              wk_sb = wpp.tile([128, NCT, 128], BF16)
              nc.sync.dma_start(wk_sb,
                                wk.rearrange("(ci r) o -> r ci o", r=128))
              wq_sb = wpp.tile([128, NCT, 128], BF16)
              nc.sync.dma_start(wq_sb,
                                wq.rearrange("(ci r) o -> r ci o", r=128))
              wv_sb = wpp.tile([128, NCT, 128], BF16)
              nc.sync.dma_start(wv_sb,
                                wv.rearrange("(ci r) o -> r ci o", r=128))

              xT = xtp.tile([128, NCT, NT], BF16)
              xr = xt.rearrange("(ci p) t -> p ci t", p=128)
              for g in range(8):
                  gs = slice(g * 512, (g + 1) * 512)
                  nc.sync.dma_start(xT[:, :, gs], xr[:, :, gs])

              # ones column of v_aug (softmax denominator accumulator)
              nc.vector.memset(vaug[:, :, :, HD:HD + 1], 1.0)
              nc.vector.memset(q2a[HD:128, :], 0.0)
              nc.vector.memset(q2b[0:HD, :], 0.0)

              for tt in range(NT // TCH):
                  ts = slice(tt * TCH, (tt + 1) * TCH)
                  ps = qkps.tile([128, TCH], F32, tag="qkp")
                  for ci in range(NCT):
                      nc.tensor.matmul(ps, wk_sb[:, ci, :], xT[:, ci, ts],
                                       start=(ci == 0), stop=(ci == NCT - 1))
                  nc.scalar.activation(k2[:, ts], ps, AF.Identity,
                                       bias=bqkv_sb[:, 1:2])
                  ps = qkps.tile([128, TCH], F32, tag="qkp")
                  for ci in range(NCT):
                      nc.tensor.matmul(ps, wq_sb[:, ci, :], xT[:, ci, ts],
                                       start=(ci == 0), stop=(ci == NCT - 1))
                  nc.scalar.activation(q2a[0:HD, ts], ps[0:HD, :], AF.Identity,
                                       bias=bqkv_sb[0:HD, 0:1])
                  nc.scalar.activation(q2b[HD:128, ts], ps[HD:128, :],
                                       AF.Identity, bias=bqkv_sb[HD:128, 0:1])
                  # v: feature-major matmul, bias, then PE transpose into
                  # the token-major v_aug layout
                  ps = qkps.tile([128, TCH], F32, tag="qkp")
                  for ci in range(NCT):
                      nc.tensor.matmul(ps, wv_sb[:, ci, :], xT[:, ci, ts],
                                       start=(ci == 0), stop=(ci == NCT - 1))
                  vt = vtp.tile([128, TCH], BF16, tag="vt")
                  nc.scalar.activation(vt, ps, AF.Identity,
                                       bias=bqkv_sb[:, 2:3])
                  for u in range(4):
                      kt = 4 * tt + u
                      tp = vtps.tile([128, 128], BF16, tag="tp")
                      nc.tensor.transpose(
                          tp, vt[:, u * 128:(u + 1) * 128], ident_bf)
                      nc.vector.tensor_copy(
                          vaug[:, kt, :, 0:HD],
                          tp[:].rearrange("p (h f) -> p h f", f=HD))

          # ================= Phase 2: causal attention ===================
          with ExitStack() as S3:
              pup = S3.enter_context(tc.tile_pool(name="pup", bufs=3))
              avsp = S3.enter_context(tc.tile_pool(name="avsp", bufs=3))
              stps = S3.enter_context(tc.tile_pool(name="stps", bufs=3,
                                                   space="PSUM"))
              avps = S3.enter_context(tc.tile_pool(name="avps", bufs=2,
                                                   space="PSUM"))

              for hl in range(2):
                  qh = q2a if hl == 0 else q2b
                  a2a_i = a2a_in1 if hl == 0 else a2a_in2
                  for b in range(B):
                      for j in (3, 2, 1, 0):  # quads, biggest first
                          qb = b * T + j * 512
                          nki = 4 * j + 4
                          av = avps.tile([HD + 1, TCH], F32, tag="av")
                          for p in range(nki // 2):
                              ki0, ki1 = 2 * p, 2 * p + 1
                              su = max(0, ki0 - 4 * j) * 128
                              st = stps.tile([128, 2, TCH], F32, tag="st")
                              for sl, ki in ((0, ki0), (1, ki1)):
                                  # write from the pair's union start so the
                                  # batched exp below never reads stale PSUM
                                  # (exp(garbage) can be inf/NaN)
                                  ks = slice(b * T + ki * 128,
                                             b * T + ki * 128 + 128)
                                  nc.tensor.matmul(
                                      st[:, sl, su:TCH],
                                      k2[:, ks],
                                      qh[:, qb + su:qb + TCH],
                                      start=True, stop=True)
                              pu = pup.tile([128, 2, TCH], BF16, tag="pu")
                              nc.scalar.activation(pu[:, :, su:TCH],
                                                   st[:, :, su:TCH],
                                                   AF.Exp, scale=SCALE)
                              for sl, ki in ((0, ki0), (1, ki1)):
                                  if ki >= 4 * j:  # diagonal tile
                                      s = (ki - 4 * j) * 128
                                      nc.gpsimd.tensor_mul(
                                          pu[:, sl, s:s + 128],
                                          pu[:, sl, s:s + 128], diag_sb)
                              for sl, ki in ((0, ki0), (1, ki1)):
                                  s = max(0, ki - 4 * j) * 128
                                  nc.tensor.matmul(
                                      av[:, s:TCH],
                                      vaug[:, b * NQT + ki, hl, :],
                                      pu[:, sl, s:TCH],
                                      start=(ki == 0), stop=(ki == nki - 1),
                                      skip_group_check=True)
                          avs = avsp.tile([HD + 1, TCH], F16, tag="avs")
                          nc.vector.tensor_copy(avs, av)
                          d = 4 * b + j
                          nc.sync.dma_start(a2a_i[d, :, :], avs)
                  if collective:
                      nc.gpsimd.collective_compute(
                          "AllToAll", mybir.AluOpType.bypass,
                          replica_groups=[list(range(N_CORES))],
                          ins=[(a2a_in1 if hl == 0 else a2a_in2)[:].opt()],
                          outs=[(a2a_out1 if hl == 0 else a2a_out2)[:].opt()])
                  else:
                      nc.sync.dma_start(
                          (a2a_out1 if hl == 0 else a2a_out2)[:],
                          (a2a_in1 if hl == 0 else a2a_in2)[:])

      # ================= Phase 3: normalize + h = a + LN(a) =============
      with ExitStack() as SH:
          hp = SH.enter_context(tc.tile_pool(name="hp", bufs=1))
          aT = hp.tile([128, NCT, TCH], F16)
          hT = hp.tile([128, NCT, TCH], BF16)
          fT = hp.tile([128, NCT, TCH], F32R)

          with ExitStack() as S4:
              arp = S4.enter_context(tc.tile_pool(name="arp", bufs=1))
              sqp = S4.enter_context(tc.tile_pool(name="sqp", bufs=3))
              stsb = S4.enter_context(tc.tile_pool(name="stsb", bufs=1))
              dnps = S4.enter_context(tc.tile_pool(name="dnps", bufs=2,
                                                   space="PSUM"))
              smps = S4.enter_context(tc.tile_pool(name="smps", bufs=1,
                                                   space="PSUM"))

              araw = arp.tile([128, NCT, TCH], F16)
              sum_ps = smps.tile([1, TCH], F32, tag="sma")
              sq_ps = smps.tile([1, TCH], F32, tag="smb")
              rcp8 = [None, None]
              # half 0 depends only on the first AllToAll and overlaps the
              # second; half 1 follows once a2a_out2 lands
              for h, a2o in ((0, a2a_out1), (1, a2a_out2)):
                  hs = slice(64 * h, 64 * h + 64)
                  nc.sync.dma_start(
                      araw[hs, :, :],
                      a2o[:, 0:HD, :].rearrange("c p t -> p c t"))
                  den8 = arp.tile([NCT, TCH], F16, tag=f"d8{h}")
                  nc.sync.dma_start(
                      den8, a2o[:, HD:HD + 1, :].rearrange("c p t -> (c p) t"))
                  r8 = arp.tile([NCT, TCH], F32R, tag=f"r8{h}")
                  nc.vector.reciprocal(r8, den8)
                  rcp8[h] = r8
              for h in range(2):
                  hs = slice(64 * h, 64 * h + 64)
                  pick = pick8a if h == 0 else pick8b
                  for ci in range(NCT):
                      dbc = dnps.tile([128, TCH], F32, tag="dbc")
                      nc.tensor.matmul(dbc, pick[:, ci, :], rcp8[h],
                                       start=True, stop=True)
                      nc.vector.tensor_mul(aT[hs, ci, :], araw[hs, ci, :],
                                           dbc[hs, :])
                      nc.tensor.matmul(sum_ps, ones16[hs, :], aT[hs, ci, :],
                                       start=(h == 0 and ci == 0),
                                       stop=(h == 1 and ci == NCT - 1),
                                       skip_group_check=True)
                      asq = sqp.tile([128, TCH], F32R, tag="asq")
                      nc.scalar.activation(asq[hs, :], aT[hs, ci, :],
                                           AF.Square)
                      nc.tensor.matmul(sq_ps, ones_all[hs, 0:1], asq[hs, :],
                                       start=(h == 0 and ci == 0),
                                       stop=(h == 1 and ci == NCT - 1),
                                       skip_group_check=True)
              mu_sb = stsb.tile([1, TCH], F32R, tag="s1")
              nc.vector.tensor_scalar_mul(mu_sb, sum_ps, 1.0 / C)
              ex2 = stsb.tile([1, TCH], F32, tag="s2")
              nc.vector.tensor_scalar_mul(ex2, sq_ps, 1.0 / C)
              musq = stsb.tile([1, TCH], F32, tag="s3")
              nc.vector.tensor_mul(musq, mu_sb, mu_sb)
              var = stsb.tile([1, TCH], F32, tag="s4")
              nc.vector.tensor_sub(var, ex2, musq)
              sd = stsb.tile([1, TCH], F32, tag="s5")
              nc.scalar.activation(sd, var, AF.Sqrt, bias=eps_sb[0:1, :])
              rs_sb = stsb.tile([1, TCH], F32R, tag="s6")
              nc.vector.reciprocal(rs_sb, sd)
              mu_bc = dnps.tile([128, TCH], F32, tag="dbc")
              nc.tensor.matmul(mu_bc, ones_all[0:1, :], mu_sb[:],
                               start=True, stop=True)
              rs_bc = dnps.tile([128, TCH], F32, tag="dbc")
              nc.tensor.matmul(rs_bc, ones_all[0:1, :], rs_sb[:],
                               start=True, stop=True)
              mu_bs = sqp.tile([128, TCH], F32, tag="mbs")
              nc.vector.tensor_copy(mu_bs, mu_bc)
              rs_bs = sqp.tile([128, TCH], F32, tag="rbs")
              nc.vector.tensor_copy(rs_bs, rs_bc)
              for ci in range(NCT):
                  t1 = sqp.tile([128, TCH], F32, tag="t1")
                  nc.gpsimd.tensor_sub(t1, aT[:, ci, :], mu_bs)
                  t2 = sqp.tile([128, TCH], F32, tag="t2")
                  nc.gpsimd.tensor_mul(t2, t1, rs_bs)
                  nc.vector.scalar_tensor_tensor(
                      out=hT[:, ci, :], in0=t2,
                      scalar=lnw_sb[:, ci:ci + 1], in1=aT[:, ci, :],
                      op0=ALU.mult, op1=ALU.add)

          # ================= Phase 4/5: FFN (+ final LN stats) ==========
          with ExitStack() as S5:
              gp = S5.enter_context(tc.tile_pool(name="gp", bufs=1))
              w1p = S5.enter_context(tc.tile_pool(name="w1p", bufs=3))
              w2p = S5.enter_context(tc.tile_pool(name="w2p", bufs=2))
              sq7p = S5.enter_context(tc.tile_pool(name="sq7p", bufs=2))
              st7 = S5.enter_context(tc.tile_pool(name="st7", bufs=1))
              ffps = S5.enter_context(tc.tile_pool(name="ffps", bufs=3,
                                                   space="PSUM"))
              sm7ps = S5.enter_context(tc.tile_pool(name="sm7ps", bufs=1,
                                                    space="PSUM"))

              gT = gp.tile([128, DFF // 128, TCH], BF16)
              for mt in range(DFF // 128):
                  wt = w1p.tile([128, NCT, 128], BF16, tag="w1t")
                  nc.sync.dma_start(
                      wt, w1.rearrange("(ci r) f -> r ci f", r=128)
                      [:, :, mt * 128:(mt + 1) * 128])
                  ps = ffps.tile([128, TCH], F32, tag="f1")
                  for ci in range(NCT):
                      nc.tensor.matmul(ps, wt[:, ci, :], hT[:, ci, :],
                                       start=(ci == 0), stop=(ci == NCT - 1))
                  nc.scalar.activation(gT[:, mt, :], ps, AF.Relu,
                                       bias=b1_sb[:, mt:mt + 1])

              sum7 = sm7ps.tile([1, TCH], F32, tag="sm7a")
              sq7 = sm7ps.tile([1, TCH], F32, tag="sm7b")
              for ci in range(NCT):
                  wt = w2p.tile([128, DFF // 128, 128], BF16, tag="w2t")
                  nc.sync.dma_start(
                      wt, w2.rearrange("(gk r) f -> r gk f", r=128)
                      [:, :, ci * 128:(ci + 1) * 128])
                  ps = ffps.tile([128, TCH], F32, tag="f2")
                  for gk in range(DFF // 128):
                      nc.tensor.matmul(ps, wt[:, gk, :], gT[:, gk, :],
                                       start=(gk == 0),
                                       stop=(gk == DFF // 128 - 1))
                  nc.scalar.activation(fT[:, ci, :], ps, AF.Identity,
                                       bias=b2_sb[:, ci:ci + 1])
                  nc.tensor.matmul(sum7, ones_all[:, 0:1], fT[:, ci, :],
                                   start=(ci == 0), stop=(ci == NCT - 1))
                  asq7 = sq7p.tile([128, TCH], F32R, tag="asq7")
                  nc.scalar.activation(asq7, fT[:, ci, :], AF.Square)
                  nc.tensor.matmul(sq7, ones_all[:, 0:1], asq7[:],
                                   start=(ci == 0), stop=(ci == NCT - 1))

              # ---- final LN stats + out = f + LN(f), feature-major
              with ExitStack() as S7:
                  op7 = S7.enter_context(tc.tile_pool(name="op7", bufs=2))

                  mu7 = st7.tile([1, TCH], F32R, tag="m7")
                  nc.vector.tensor_scalar_mul(mu7, sum7, 1.0 / C)
                  ex27 = st7.tile([1, TCH], F32, tag="e7")
                  nc.vector.tensor_scalar_mul(ex27, sq7, 1.0 / C)
                  musq7 = st7.tile([1, TCH], F32, tag="mq7")
                  nc.vector.tensor_mul(musq7, mu7, mu7)
                  var7 = st7.tile([1, TCH], F32, tag="v7")
                  nc.vector.tensor_sub(var7, ex27, musq7)
                  sd7 = st7.tile([1, TCH], F32, tag="sd7")
                  nc.scalar.activation(sd7, var7, AF.Sqrt,
                                       bias=eps_sb[0:1, :])
                  rs7 = st7.tile([1, TCH], F32R, tag="rs7")
                  nc.vector.reciprocal(rs7, sd7)
                  mu7_bc = ffps.tile([128, TCH], F32, tag="f2")
                  nc.tensor.matmul(mu7_bc, ones_all[0:1, :], mu7[:],
                                   start=True, stop=True)
                  rs7_bc = ffps.tile([128, TCH], F32, tag="f2")
                  nc.tensor.matmul(rs7_bc, ones_all[0:1, :], rs7[:],
                                   start=True, stop=True)
                  mu7_bs = op7.tile([128, TCH], F32, tag="m7s")
                  nc.vector.tensor_copy(mu7_bs, mu7_bc)
                  rs7_bs = op7.tile([128, TCH], F32, tag="r7s")
                  nc.vector.tensor_copy(rs7_bs, rs7_bc)
                  ocr = outc.rearrange("(ci p) t -> p ci t", p=128)
                  for ci in range(NCT):
                      t1 = op7.tile([128, TCH], F32, tag="t17")
                      nc.vector.tensor_sub(t1, fT[:, ci, :], mu7_bs)
                      t2 = op7.tile([128, TCH], F32, tag="t27")
                      nc.gpsimd.tensor_mul(t2, t1, rs7_bs)
                      t3 = op7.tile([128, TCH], F32, tag="t37")
                      nc.scalar.activation(t3, t2, AF.Identity,
                                           scale=lnw_sb[:, ci:ci + 1])
                      ot = op7.tile([128, TCH], F32, tag="ot7")
                      nc.vector.tensor_add(ot, t3, fT[:, ci, :])
                      nc.sync.dma_start(ocr[:, ci, :], ot)

    nc.compile()
    return nc


def _stage(inputs):
    bf = ml_dtypes.bfloat16
    f16 = np.float16
    x = np.asarray(inputs["x"], dtype=np.float32)
    xt = np.ascontiguousarray(x.reshape(NT, C).T.astype(bf))
    Wq = np.asarray(inputs["Wq"], np.float32)
    Wk = np.asarray(inputs["Wk"], np.float32)
    Wv = np.asarray(inputs["Wv"], np.float32)
    bq = np.asarray(inputs["bq"], np.float32)
    bk = np.asarray(inputs["bk"], np.float32)
    bv = np.asarray(inputs["bv"], np.float32)

    diag = np.triu(np.ones((128, 128), np.float32)).astype(bf)
    pick8a = np.zeros((8, NCT, 128), np.float32)
    pick8b = np.zeros((8, NCT, 128), np.float32)
    for ci in range(NCT):
        pick8a[ci, ci, 0:64] = 1.0
        pick8b[ci, ci, 64:128] = 1.0

    shared = {
        "xt": xt,
        "w1": np.ascontiguousarray(np.asarray(inputs["W1"], np.float32).astype(bf)),
        "w2": np.ascontiguousarray(np.asarray(inputs["W2"], np.float32).astype(bf)),
        "b1_r": np.ascontiguousarray(
            np.asarray(inputs["b1"], np.float32).reshape(DFF // 128, 128).T),
        "b2_r": np.ascontiguousarray(
            np.asarray(inputs["b2"], np.float32).reshape(NCT, 128).T),
        "lnw_r": np.ascontiguousarray(
            np.asarray(inputs["ln_w"], np.float32).reshape(NCT, 128).T),
        "ident_in": np.eye(128, dtype=np.float32),
        "ones_in": np.ones((128, 128), dtype=np.float32),
        "diag_in": np.ascontiguousarray(diag),
        "pick8a_in": pick8a,
        "pick8b_in": pick8b,
    }
    in_maps = []
    for c in range(N_CORES):
        sl = slice(128 * c, 128 * c + 128)
        per = dict(shared)
        per["wq"] = np.ascontiguousarray(Wq[:, sl].astype(bf))
        per["wk"] = np.ascontiguousarray(Wk[:, sl].astype(bf))
        per["wv"] = np.ascontiguousarray(Wv[:, sl].astype(bf))
        per["bqkv"] = np.ascontiguousarray(
            np.stack([bq[sl], bk[sl], bv[sl]], axis=1).astype(np.float32))
        in_maps.append(per)
    return in_maps


def kernel(**inputs):
    from concourse.bass_utils import run_bass_kernel_spmd

    nc = _CACHE.get("nc")
    if nc is None:
        nc = _CACHE["nc"] = _build()
    in_maps = _stage(inputs)
    res = run_bass_kernel_spmd(nc, in_maps, core_ids=list(range(N_CORES)))
    out = np.empty((B, T, C), dtype=np.float32)
    for c in range(N_CORES):
        b, m = divmod(c, 4)
        out[b, m * TCH:(m + 1) * TCH, :] = res.results[c]["outc"].T
    return out
